# revision 15
# baseline (speedup 1.0000x reference)
"""Trainium2 Bass kernel for nn_ConvPlus1d (dense_cnn).

Algorithm (mathematically identical to the reference, derived analytically):

  The reference synthesizes per-sample conv weights:
      kern[b]   = mean_L(depthwise_conv(x))        -> [B, C_IN, K]
      w_in[b]   = W_in @ kern[b]                   -> [B, C_IN, K]
      w_out[b]  = <W_out, kern[b]>                 -> [B, C_OUT]
      bias[b]   = <W_bias, kern[b]>                -> [B, C_OUT]
      weight[b, o, c, k] = w_in[b, c, k] * w_out[b, o]     (rank-1!)
      y[b] = conv1d(x[b], weight[b], pad=1) + bias[b]

  Exact simplifications:
  1) mean over L of a pad-1 depthwise conv only needs per-channel sums and
     the first/last elements:  sum_l xpad[c, l+t] = {S-E, S, S-F}[t]
     so kern / w_in / w_out / bias are LINEAR in (S, E, F), with
     coefficient matrices precomputed on the host from maker params.
  2) The per-sample conv weight is rank-1 across (o) x (c,k).

  Device program per sample (data-parallel over batch, 4 samples/core):
      x (bf16) lands in SBUF partitions 0-63; a shifted copy (one column
      left) is DMA'd into partitions 64-127.  The 3-tap conv then needs
      only TWO matmuls per 512-col tile: a 128-contract matmul computes
      taps 0+1 together (stationary [W0; W1]), a 64-contract matmul adds
      tap 2, rolling over 7 PSUM banks so consecutive matmuls hit
      different banks and the PE pipeline never drains (keeps the tensor
      engine p-state ramping toward 2.4 GHz; a burst of warm-up matmuls
      at t~3us starts the ramp before the first x chunk even lands).
      Stats (per-channel sums) are spread across DVE and GPSIMD (plus
      ACT's activation-accumulator at the head), and -- like the synth
      steps -- are dropped into the conv instruction stream as deferred
      closures so no engine's in-order queue ever stalls the pipeline.
      PSUM -> SBUF eviction adds the bias and narrows to bf16 (output
      is stored bf16 and widened to f32 on the host: the conv itself
      runs in bf16 so this costs ~1e-3 extra relative error but halves
      store traffic).  Stores stream out in 1024-col chunks on the Sync
      engine's hardware DGE queue.

Sharding: batch 32 -> 8 cores x 4 samples, maker params replicated.
"""

import sys

import numpy as np

sys.path.insert(0, "/opt/trn_rl_repo")

import concourse.bacc as bacc  # noqa: E402
import concourse.tile as tile  # noqa: E402
from concourse import mybir  # noqa: E402
from concourse.bass_utils import run_bass_kernel_spmd  # noqa: E402

import ml_dtypes  # noqa: E402

B, C_IN, C_OUT, K, L = 32, 64, 128, 3, 8192
N_CORES = 8
BS = B // N_CORES          # samples per core
NT = 512                   # matmul moving-dim tile (one PSUM bank of fp32)
NTILES = L // NT

F32 = mybir.dt.float32
F32R = mybir.dt.float32r
BF16 = mybir.dt.bfloat16

IDENT = mybir.ActivationFunctionType.Identity


def _host_precompute(W_kernel, W_in, W_out, W_bias):
    """Fold the maker parameters into linear maps on the stats (S, E, F)."""
    Wk = W_kernel.reshape(C_IN, K, K).astype(np.float64)     # [c, j, t]
    P = (Wk[:, :, 0] + Wk[:, :, 1] + Wk[:, :, 2]) / L        # coeff on S
    Q = -Wk[:, :, 0] / L                                     # coeff on E
    R = -Wk[:, :, 2] / L                                     # coeff on F

    Win = W_in[:, :, 0].astype(np.float64)                   # [c, c']

    def m_in(Xc):   # -> [c', k*64+c]
        return np.einsum("cp,pk->pkc", Win, Xc).reshape(C_IN, K * C_IN)

    def m_out(Xc, W):  # -> [c', o]
        return np.einsum("ock,ck->co", W.astype(np.float64), Xc)

    def mm(Xc):
        return np.concatenate([m_in(Xc), m_out(Xc, W_out)], axis=1)  # [64,320]

    m3 = np.stack([mm(P), mm(Q), mm(R)], axis=1)             # [64, 3, 320]
    mb3 = np.stack(
        [m_out(P, W_bias), m_out(Q, W_bias), m_out(R, W_bias)], axis=1
    )                                                        # [64, 3, 128]
    return m3.astype(np.float32), mb3.astype(np.float32)


_CACHE = {}

# stats windows for samples 1-3 (one big load): 8 windows.  Both
# zero-pad columns are included: they add nothing to S.
_W8 = (0, 1024, 2048, 3072, 4096, 5120, 6144, 7168, L + 2)
_ENGS8 = ("v", "v", "a", "v", "v", "a", "v", "v")


def _stats_win(nc, scr, xh, Sp, col, lo, hi, eng):
    """One partial-sum window on the chosen engine (DVE or ACT)."""
    if eng == "v":
        nc.vector.reduce_sum(out=Sp[:, col:col + 1],
                             in_=xh[0:C_IN, lo:hi],
                             axis=mybir.AxisListType.X)
    else:
        # ACT reduces via the activation accumulator; the full-size out
        # write is a throwaway scratch.
        sc = scr.tile([C_IN, 2048], BF16, tag="scr")
        nc.scalar.activation(sc[:, 0:hi - lo], xh[0:C_IN, lo:hi], IDENT,
                             accum_out=Sp[:, col:col + 1])


def _emit_stage(nc, xp, small, scr, x_d, b, defer=False):
    """Issue x load + shifted copy for sample b; stats either emitted
    now (head samples) or returned as deferred closures to drop into an
    earlier sample's conv stream (so DVE/GPSIMD's in-order queues never
    stall behind a load that has not landed yet)."""
    xh = xp.tile([2 * C_IN, L + 2], BF16, tag="xh")
    Sp = small.tile([C_IN, 8], F32, tag="Sp")
    closures = []
    if b == 0:
        # sample 0 sits on the critical ramp: chunk the load/copy so the
        # stats reduces pipeline behind the DMA, and fan the reduces out
        # across three engines so the last window finishes ~1us after
        # the last chunk lands.
        H = 4097
        lb = (0, 2048, H, 6144, L + 2)
        for c in range(4):
            nc.sync.dma_start(xh[0:C_IN, lb[c]:lb[c + 1]],
                              x_d[b][:, lb[c]:lb[c + 1]])
        for c in range(4):
            d0, d1 = max(lb[c] - 1, 0), lb[c + 1] - 1
            nc.sync.dma_start(xh[C_IN:, d0:d1], xh[0:C_IN, d0 + 1:d1 + 1])
        wins = ((0, 2048, "v"), (2048, 4096, "a"), (4096, 6144, "v"),
                (6144, 7169, "v"), (7169, L + 2, "a"))
        for c, (lo, hi, eng) in enumerate(wins):
            _stats_win(nc, scr, xh, Sp, c, lo, hi, eng)
        ncols = 5
    else:
        nc.sync.dma_start(xh[0:C_IN, :], x_d[b])
        # dest col j <- src col j+1 (the shifted copy for taps 1/2)
        nc.sync.dma_start(xh[C_IN:, 0:L + 1], xh[0:C_IN, 1:L + 2])

        def win(c):
            return lambda: _stats_win(nc, scr, xh, Sp, c, _W8[c],
                                      _W8[c + 1], _ENGS8[c])

        if defer:
            closures = [win(c) for c in range(8)]
        else:
            for c in range(8):
                win(c)()
        ncols = 8
    return (xh, Sp, ncols), closures


def _emit_synth_steps(nc, small, pss, m3, mb3, xh, Sp, ncols):
    """Stats -> (w01, w2, biasv) for one sample, as four deferred steps.

    The steps are interleaved into the PREVIOUS sample's conv matmul
    stream so the PE <-> DVE ping-pong never drains the tensor engine
    (which would also drop its p-state).  All synth PSUM packs into ONE
    bank (disjoint partition/col ranges, so interleaved accumulation
    groups are safe: skip_group_check), leaving 7 banks for the conv.
    """
    stat = small.tile([C_IN, 3], F32R, tag="stat")
    # ps01 cols 0-127 (all partitions), ps2 cols 128-255 (partitions
    # 64-127), psp cols 128-447 (partition 0 only -- disjoint partitions
    # from ps2), psb col 448.
    syn = pss.tile([2 * C_IN, 512], F32, tag="syn")
    ps01, ps2 = syn[:, 0:128], syn[C_IN:, 128:256]
    psp, psb = syn[0:1, 128:448], syn[:, 448:449]
    params = small.tile([1, 320], F32R, tag="params")
    biasv = small.tile([C_OUT, 1], F32, tag="biasv")
    w01 = small.tile([2 * C_IN, C_OUT], BF16, tag="w01")
    w2 = small.tile([2 * C_IN, C_OUT], BF16, tag="w2")

    def step0():   # stats gather (DVE) + stat matmuls (PE)
        # fp32r is 32-bit in SBUF: the low-precision guard is a false alarm
        with nc.allow_low_precision(reason="fp32r out is fp32 bits"):
            nc.vector.reduce_sum(out=stat[:, 0:1], in_=Sp[:, 0:ncols],
                                 axis=mybir.AxisListType.X)
        nc.vector.tensor_copy(stat[:, 1:2], xh[0:C_IN, L:L + 1])   # E
        nc.vector.tensor_copy(stat[:, 2:3], xh[0:C_IN, 1:2])       # F
        for j in range(3):
            sj = stat[:, j:j + 1]
            nc.tensor.matmul(psp, sj, m3[:, j, :], start=(j == 0),
                             stop=(j == 2), skip_group_check=True)
            # 1 moving column: fp32 4-pass costs nothing, and fp32r
            # moving free-size 1 fails the ISA check
            nc.tensor.matmul(psb, mb3[:, j, :], sj.bitcast(F32),
                             start=(j == 0), stop=(j == 2),
                             skip_group_check=True)

    def step1():
        nc.vector.tensor_copy(params[:], psp)
        nc.vector.tensor_copy(biasv[:], psb)

    def step2():
        # rank-1 stationaries: [W0; W1] on partitions 0-127, W2 on
        # 64-127.  contract-1 outers: fp32r fails the ISA check, fp32
        # 4-pass on 128 moving cols is well under a microsecond.
        pr = params[0:1].bitcast(F32)
        w_out_row = pr[:, 192:320]
        nc.tensor.matmul(ps01, pr[:, 0:128], w_out_row, start=True,
                         stop=True, skip_group_check=True)
        nc.tensor.matmul(ps2, pr[:, 128:192], w_out_row, start=True,
                         stop=True, skip_group_check=True)

    def step3():
        nc.vector.tensor_copy(w01[:], ps01)
        nc.vector.tensor_copy(w2[C_IN:, :], ps2)

    return (w01, w2, biasv), [step0, step1, step2, step3]


def _emit_conv(nc, yp, yps, y_d, b, xh, w01, w2, biasv, closures=()):
    """Main conv for one sample, rolling software pipeline over tile
    PAIRS.  yps is one persistent [C_OUT, 3, 1024] f32 PSUM tile (six
    banks, three double-wide rotation slots): consecutive PE matmuls
    always hit different banks so the PE never waits on an accumulate
    turnaround, and each eviction drains 1024 cols in one instruction
    (f32 PSUM APs may span two adjacent banks).  Evictions alternate
    ACT / DVE so neither engine's in-order queue gates the bank
    rotation.  `closures` are the next sample's synth stages plus later
    samples' deferred stats, dropped in one per tile pair."""
    ysbs = [None] * (NTILES // 2)

    def mm1(t):
        d = (t // 2) % 3
        py = yps[:, d, (t % 2) * NT:(t % 2) * NT + NT]
        nc.tensor.matmul(py, w01[:], xh[:, NT * t:NT * t + NT],
                         start=True, stop=False, skip_group_check=True)

    def mm2(t):
        d = (t // 2) % 3
        py = yps[:, d, (t % 2) * NT:(t % 2) * NT + NT]
        nc.tensor.matmul(py, w2[C_IN:, :],
                         xh[C_IN:, NT * t + 1:NT * t + NT + 1],
                         start=False, stop=True, skip_group_check=True)

    def evict(h):
        d = h % 3
        ysbs[h] = yp.tile([C_OUT, 2 * NT], BF16, name="ysb", tag="ysb")
        if h % 2 == 0:
            nc.scalar.activation(ysbs[h][:], yps[:, d, :], IDENT,
                                 bias=biasv[:], scale=1.0)
        else:
            nc.vector.tensor_scalar(ysbs[h][:], yps[:, d, :], biasv[:],
                                    None, mybir.AluOpType.add)

    def store(h):
        nc.sync.dma_start(y_d[b][:, 2 * NT * h:2 * NT * (h + 1)],
                          ysbs[h][:])

    ci = iter(closures)
    for h in range(NTILES // 2):
        mm1(2 * h)
        mm1(2 * h + 1)
        mm2(2 * h)
        mm2(2 * h + 1)
        evict(h)
        store(h)
        if h >= 1:
            for c in (next(ci, None), next(ci, None)):
                if c is not None:
                    c()
    for c in ci:       # anything left over
        c()


def _build_module():
    if "nc" in _CACHE:
        return _CACHE["nc"]
    nc = bacc.Bacc("TRN2", target_bir_lowering=False, debug=False)

    # host supplies x pre-padded with one zero column on each side, bf16
    x_d = nc.dram_tensor("x", [BS, C_IN, L + 2], BF16,
                         kind="ExternalInput").ap()
    m3_d = nc.dram_tensor("m3", [C_IN, 3, 320], F32R,
                          kind="ExternalInput").ap()
    mb3_d = nc.dram_tensor("mb3", [C_IN, 3, C_OUT], F32,
                           kind="ExternalInput").ap()
    y_d = nc.dram_tensor("y", [BS, C_OUT, L], BF16,
                         kind="ExternalOutput").ap()

    with tile.TileContext(nc) as tc:
        with (
            tc.tile_pool(name="consts", bufs=1) as consts,
            tc.tile_pool(name="xp", bufs=4) as xp,
            tc.tile_pool(name="yp", bufs=6) as yp,
            tc.tile_pool(name="small", bufs=2) as small,
            tc.tile_pool(name="scr", bufs=2) as scr,
            tc.tile_pool(name="ps_y", bufs=1, space="PSUM") as psyp,
            tc.tile_pool(name="ps_s", bufs=1, space="PSUM") as pss,
        ):
            m3 = consts.tile([C_IN, 3, 320], F32R)
            mb3 = consts.tile([C_IN, 3, C_OUT], F32)
            # six PSUM banks as three double-wide conv rotation slots
            yps = psyp.tile([C_OUT, 3, 2 * NT], F32)

            # PE warm-up: a burst of throwaway matmuls starting as soon
            # as the memset lands (~3us, before any DMA arrives) keeps
            # the tensor engine continuously busy so its p-state ramps
            # toward 2.4 GHz before the real conv stream begins.
            wsrc = consts.tile([2 * C_IN, NT], BF16)
            nc.gpsimd.memset(wsrc[:], 0.0)
            for _ in range(20):
                nc.tensor.matmul(yps[0:1, 0, 0:NT], wsrc[:, 0:1], wsrc[:],
                                 start=True, stop=True,
                                 skip_group_check=True)

            # software pipeline: stage(b) issues loads/copies, synth(b)
            # runs the small fp32r matmul chain, conv(b) the 16-tile
            # conv.  stage(b+2)'s loads are issued before conv(b) so its
            # DMAs sit ahead of conv(b)'s stores in the queues, but its
            # stats closures are dropped INTO conv(b)'s stream.
            stages = {}
            stages[0], _ = _emit_stage(nc, xp, small, scr, x_d, 0)
            nc.scalar.dma_start(m3[:], m3_d)
            nc.scalar.dma_start(mb3[:], mb3_d)
            tiles0, steps0 = _emit_synth_steps(nc, small, pss, m3, mb3,
                                               *stages[0])
            for s in steps0:           # sample 0: run synth immediately
                s()
            stages[1], _ = _emit_stage(nc, xp, small, scr, x_d, 1)
            synth = {0: tiles0}
            for b in range(BS):
                stats_cl = []
                if b + 2 < BS:
                    stages[b + 2], stats_cl = _emit_stage(
                        nc, xp, small, scr, x_d, b + 2, defer=True)
                if b + 1 < BS:
                    synth[b + 1], steps = _emit_synth_steps(
                        nc, small, pss, m3, mb3, *stages[b + 1])
                else:
                    steps = []
                # interleave: synth steps early (they gate the next
                # conv), stats sprinkled between them.
                closures = []
                si, wi = iter(steps), iter(stats_cl)
                order = "swswswswwwwwww"
                for ch in order:
                    c = next(si if ch == "s" else wi, None)
                    if c is not None:
                        closures.append(c)
                _emit_conv(nc, yp, yps, y_d, b, stages[b][0], *synth[b],
                           closures=closures)

    nc.compile()
    _CACHE["nc"] = nc
    return nc


def kernel(x, W_kernel, W_in, W_out, W_bias):
    x = np.asarray(x, dtype=np.float32)
    # one zero column each side: the device reads x[l-1], x[l], x[l+1]
    x = np.pad(x, [(0, 0), (0, 0), (1, 1)]).astype(ml_dtypes.bfloat16)
    m3, mb3 = _host_precompute(
        np.asarray(W_kernel, np.float32), np.asarray(W_in, np.float32),
        np.asarray(W_out, np.float32), np.asarray(W_bias, np.float32))

    nc = _build_module()
    in_maps = [
        {"x": x[c * BS:(c + 1) * BS], "m3": m3, "mb3": mb3}
        for c in range(N_CORES)
    ]
    res = run_bass_kernel_spmd(nc, in_maps, core_ids=list(range(N_CORES)))
    global LAST_RESULT
    LAST_RESULT = res
    y = np.concatenate([np.asarray(r["y"]) for r in res.results],
                       axis=0).astype(np.float32)
    return y


LAST_RESULT = None


# revision 19
# speedup vs baseline: 1.0508x; 1.0508x over previous
"""Trainium2 Bass kernel for nn_ConvPlus1d (dense_cnn).

Algorithm (mathematically identical to the reference, derived analytically):

  The reference synthesizes per-sample conv weights:
      kern[b]   = mean_L(depthwise_conv(x))        -> [B, C_IN, K]
      w_in[b]   = W_in @ kern[b]                   -> [B, C_IN, K]
      w_out[b]  = <W_out, kern[b]>                 -> [B, C_OUT]
      bias[b]   = <W_bias, kern[b]>                -> [B, C_OUT]
      weight[b, o, c, k] = w_in[b, c, k] * w_out[b, o]     (rank-1!)
      y[b] = conv1d(x[b], weight[b], pad=1) + bias[b]

  Exact simplifications:
  1) mean over L of a pad-1 depthwise conv only needs per-channel sums and
     the first/last elements:  sum_l xpad[c, l+t] = {S-E, S, S-F}[t]
     so kern / w_in / w_out / bias are LINEAR in (S, E, F), with
     coefficient matrices precomputed on the host from maker params.
  2) The per-sample conv weight is rank-1 across (o) x (c,k).

  Device program per sample (data-parallel over batch, 4 samples/core):
      x (bf16) lands in SBUF partitions 0-63; a shifted copy (one column
      left) is DMA'd into partitions 64-127.  The 3-tap conv then needs
      only TWO matmuls per 512-col tile: a 128-contract matmul computes
      taps 0+1 together (stationary [W0; W1]), a 64-contract matmul adds
      tap 2, rolling over 7 PSUM banks so consecutive matmuls hit
      different banks and the PE pipeline never drains (keeps the tensor
      engine p-state ramping toward 2.4 GHz; a burst of warm-up matmuls
      at t~3us starts the ramp before the first x chunk even lands).
      Stats (per-channel sums) are spread across DVE and GPSIMD (plus
      ACT's activation-accumulator at the head), and -- like the synth
      steps -- are dropped into the conv instruction stream as deferred
      closures so no engine's in-order queue ever stalls the pipeline.
      PSUM -> SBUF eviction adds the bias and narrows to bf16 (output
      is stored bf16 and widened to f32 on the host: the conv itself
      runs in bf16 so this costs ~1e-3 extra relative error but halves
      store traffic).  Stores stream out in 1024-col chunks on the Sync
      engine's hardware DGE queue.

Sharding: batch 32 -> 8 cores x 4 samples, maker params replicated.
"""

import sys

import numpy as np

sys.path.insert(0, "/opt/trn_rl_repo")

import concourse.bacc as bacc  # noqa: E402
import concourse.tile as tile  # noqa: E402
from concourse import mybir  # noqa: E402
from concourse.bass_utils import run_bass_kernel_spmd  # noqa: E402

import ml_dtypes  # noqa: E402

B, C_IN, C_OUT, K, L = 32, 64, 128, 3, 8192
N_CORES = 8
BS = B // N_CORES          # samples per core
NT = 512                   # matmul moving-dim tile (one PSUM bank of fp32)
NTILES = L // NT

F32 = mybir.dt.float32
F32R = mybir.dt.float32r
BF16 = mybir.dt.bfloat16

IDENT = mybir.ActivationFunctionType.Identity


def _host_precompute(W_kernel, W_in, W_out, W_bias):
    """Fold the maker parameters into linear maps on the stats (S, E, F)."""
    Wk = W_kernel.reshape(C_IN, K, K).astype(np.float64)     # [c, j, t]
    P = (Wk[:, :, 0] + Wk[:, :, 1] + Wk[:, :, 2]) / L        # coeff on S
    Q = -Wk[:, :, 0] / L                                     # coeff on E
    R = -Wk[:, :, 2] / L                                     # coeff on F

    Win = W_in[:, :, 0].astype(np.float64)                   # [c, c']

    def m_in(Xc):   # -> [c', k*64+c]
        return np.einsum("cp,pk->pkc", Win, Xc).reshape(C_IN, K * C_IN)

    def m_out(Xc, W):  # -> [c', o]
        return np.einsum("ock,ck->co", W.astype(np.float64), Xc)

    def mm(Xc):
        return np.concatenate([m_in(Xc), m_out(Xc, W_out)], axis=1)  # [64,320]

    m3 = np.stack([mm(P), mm(Q), mm(R)], axis=1)             # [64, 3, 320]
    mb3 = np.stack(
        [m_out(P, W_bias), m_out(Q, W_bias), m_out(R, W_bias)], axis=1
    )                                                        # [64, 3, 128]
    return m3.astype(np.float32), mb3.astype(np.float32)


_CACHE = {}

# stats windows for samples 1-3 (one big load): 8 windows.  Both
# zero-pad columns are included: they add nothing to S.
_W8 = (0, 1024, 2048, 3072, 4096, 5120, 6144, 7168, L + 2)
_ENGS8 = ("v", "v", "v", "v", "v", "v", "v", "v")


def _stats_win(nc, scr, xh, Sp, col, lo, hi, eng):
    """One partial-sum window on the chosen engine (DVE or ACT)."""
    if eng == "v":
        nc.vector.reduce_sum(out=Sp[:, col:col + 1],
                             in_=xh[0:C_IN, lo:hi],
                             axis=mybir.AxisListType.X)
    else:
        # ACT reduces via the activation accumulator; the full-size out
        # write is a throwaway scratch.
        sc = scr.tile([C_IN, 2048], BF16, tag="scr")
        nc.scalar.activation(sc[:, 0:hi - lo], xh[0:C_IN, lo:hi], IDENT,
                             accum_out=Sp[:, col:col + 1])


def _emit_stage(nc, xp, small, scr, x_d, b, defer=False):
    """Issue x load + shifted copy for sample b; stats either emitted
    now (head samples) or returned as deferred closures to drop into an
    earlier sample's conv stream (so DVE/GPSIMD's in-order queues never
    stall behind a load that has not landed yet)."""
    xh = xp.tile([2 * C_IN, L + 2], BF16, tag="xh")
    Sp = small.tile([C_IN, 8], F32, tag="Sp")
    closures = []
    if b == 0:
        # sample 0 sits on the critical ramp: chunk the load/copy so the
        # stats reduces pipeline behind the DMA, and fan the reduces out
        # across three engines so the last window finishes ~1us after
        # the last chunk lands.
        H = 4097
        lb = (0, 2048, H, 6144, L + 2)
        for c in range(4):
            nc.sync.dma_start(xh[0:C_IN, lb[c]:lb[c + 1]],
                              x_d[b][:, lb[c]:lb[c + 1]])
        for c in range(4):
            d0, d1 = max(lb[c] - 1, 0), lb[c + 1] - 1
            nc.sync.dma_start(xh[C_IN:, d0:d1], xh[0:C_IN, d0 + 1:d1 + 1])
        wins = ((0, 2048, "v"), (2048, 4096, "a"), (4096, 6144, "v"),
                (6144, 7169, "v"), (7169, L + 2, "a"))
        for c, (lo, hi, eng) in enumerate(wins):
            _stats_win(nc, scr, xh, Sp, c, lo, hi, eng)
        ncols = 5
    else:
        nc.sync.dma_start(xh[0:C_IN, :], x_d[b])
        # dest col j <- src col j+1 (the shifted copy for taps 1/2)
        nc.sync.dma_start(xh[C_IN:, 0:L + 1], xh[0:C_IN, 1:L + 2])

        def win(c):
            return lambda: _stats_win(nc, scr, xh, Sp, c, _W8[c],
                                      _W8[c + 1], _ENGS8[c])

        if defer:
            closures = [win(c) for c in range(8)]
        else:
            for c in range(8):
                win(c)()
        ncols = 8
    return (xh, Sp, ncols), closures


def _emit_synth_steps(nc, small, pss, m3, mb3, xh, Sp, ncols):
    """Stats -> (w01, w2, biasv) for one sample, as four deferred steps.

    The steps are interleaved into the PREVIOUS sample's conv matmul
    stream so the PE <-> DVE ping-pong never drains the tensor engine
    (which would also drop its p-state).  All synth PSUM packs into ONE
    bank (disjoint partition/col ranges, so interleaved accumulation
    groups are safe: skip_group_check), leaving 7 banks for the conv.
    """
    stat = small.tile([C_IN, 3], F32R, tag="stat")
    # ps01 cols 0-127 (all partitions), ps2 cols 128-255 (partitions
    # 64-127), psp cols 128-447 (partition 0 only -- disjoint partitions
    # from ps2), psb col 448.
    syn = pss.tile([2 * C_IN, 512], F32, tag="syn")
    ps01, ps2 = syn[:, 0:128], syn[C_IN:, 128:256]
    psp, psb = syn[0:1, 128:448], syn[:, 448:449]
    params = small.tile([1, 320], F32R, tag="params")
    biasv = small.tile([C_OUT, 1], F32, tag="biasv")
    w01 = small.tile([2 * C_IN, C_OUT], BF16, tag="w01")
    w2 = small.tile([2 * C_IN, C_OUT], BF16, tag="w2")

    def step0():   # stats gather (DVE) + stat matmuls (PE)
        # fp32r is 32-bit in SBUF: the low-precision guard is a false alarm
        with nc.allow_low_precision(reason="fp32r out is fp32 bits"):
            nc.vector.reduce_sum(out=stat[:, 0:1], in_=Sp[:, 0:ncols],
                                 axis=mybir.AxisListType.X)
        nc.vector.tensor_copy(stat[:, 1:2], xh[0:C_IN, L:L + 1])   # E
        nc.vector.tensor_copy(stat[:, 2:3], xh[0:C_IN, 1:2])       # F
        for j in range(3):
            sj = stat[:, j:j + 1]
            nc.tensor.matmul(psp, sj, m3[:, j, :], start=(j == 0),
                             stop=(j == 2), skip_group_check=True)
            # 1 moving column: fp32 4-pass costs nothing, and fp32r
            # moving free-size 1 fails the ISA check
            nc.tensor.matmul(psb, mb3[:, j, :], sj.bitcast(F32),
                             start=(j == 0), stop=(j == 2),
                             skip_group_check=True)

    def step1():
        nc.vector.tensor_copy(params[:], psp)
        nc.vector.tensor_copy(biasv[:], psb)

    def step2():
        # rank-1 stationaries: [W0; W1] on partitions 0-127, W2 on
        # 64-127.  contract-1 outers: fp32r fails the ISA check, fp32
        # 4-pass on 128 moving cols is well under a microsecond.
        pr = params[0:1].bitcast(F32)
        w_out_row = pr[:, 192:320]
        nc.tensor.matmul(ps01, pr[:, 0:128], w_out_row, start=True,
                         stop=True, skip_group_check=True)
        nc.tensor.matmul(ps2, pr[:, 128:192], w_out_row, start=True,
                         stop=True, skip_group_check=True)

    def step3():
        nc.vector.tensor_copy(w01[:], ps01)
        nc.vector.tensor_copy(w2[C_IN:, :], ps2)

    return (w01, w2, biasv), [step0, step1, step2, step3]


def _emit_conv(nc, yp, yps, y_d, b, xh, w01, w2, biasv, closures=()):
    """Main conv for one sample: groups of 4 tiles, k-outer within the
    group (4x mm1 then 4x mm2: one LDWEIGHTS per 4 matmuls, and the
    mm2-accumulates-onto-mm1 dependency sits 4 matmuls back).  yps is
    one persistent [C_OUT, 7, 512] f32 PSUM tile (seven single-bank
    rotation slots): group g+1's banks never collide with group g's
    un-evicted banks (4 and 3 partition 7), so the PE runs two full
    groups ahead of the evictions.  Evictions live on ACT alone (DVE
    holds the stats reduces); stores go out in 1024-col chunks on the
    GpSimd software-DGE queue, away from the loads' Sync queue.
    `closures` are the next sample's synth stages plus later samples'
    deferred stats, dropped in two per group."""
    ysbs = [None] * (NTILES // 2)

    def slot(t):
        return yps[:, t % 7, :]

    def mm1(t):
        nc.tensor.matmul(slot(t), w01[:], xh[:, NT * t:NT * t + NT],
                         start=True, stop=False, skip_group_check=True)

    def mm2(t):
        nc.tensor.matmul(slot(t), w2[C_IN:, :],
                         xh[C_IN:, NT * t + 1:NT * t + NT + 1],
                         start=False, stop=True, skip_group_check=True)

    def evict(t):
        h = t // 2
        if t % 2 == 0:
            ysbs[h] = yp.tile([C_OUT, 2 * NT], BF16, name="ysb", tag="ysb")
        o = (t % 2) * NT
        nc.scalar.activation(ysbs[h][:, o:o + NT], slot(t), IDENT,
                             bias=biasv[:], scale=1.0)

    def store(h):
        nc.gpsimd.dma_start(y_d[b][:, 2 * NT * h:2 * NT * (h + 1)],
                            ysbs[h][:])

    ci = iter(closures)
    for g in range(NTILES // 4):
        for i in range(4):
            mm1(4 * g + i)
        for i in range(4):
            mm2(4 * g + i)
        for i in range(4):
            evict(4 * g + i)
        store(2 * g)
        store(2 * g + 1)
        for c in (next(ci, None), next(ci, None), next(ci, None)):
            if c is not None:
                c()
    for c in ci:       # anything left over
        c()


def _build_module():
    if "nc" in _CACHE:
        return _CACHE["nc"]
    nc = bacc.Bacc("TRN2", target_bir_lowering=False, debug=False)

    # host supplies x pre-padded with one zero column on each side, bf16
    x_d = nc.dram_tensor("x", [BS, C_IN, L + 2], BF16,
                         kind="ExternalInput").ap()
    m3_d = nc.dram_tensor("m3", [C_IN, 3, 320], F32R,
                          kind="ExternalInput").ap()
    mb3_d = nc.dram_tensor("mb3", [C_IN, 3, C_OUT], F32,
                           kind="ExternalInput").ap()
    y_d = nc.dram_tensor("y", [BS, C_OUT, L], BF16,
                         kind="ExternalOutput").ap()

    with tile.TileContext(nc) as tc:
        with (
            tc.tile_pool(name="consts", bufs=1) as consts,
            tc.tile_pool(name="xp", bufs=4) as xp,
            tc.tile_pool(name="yp", bufs=6) as yp,
            tc.tile_pool(name="small", bufs=2) as small,
            tc.tile_pool(name="scr", bufs=2) as scr,
            tc.tile_pool(name="ps_y", bufs=1, space="PSUM") as psyp,
            tc.tile_pool(name="ps_s", bufs=1, space="PSUM") as pss,
        ):
            m3 = consts.tile([C_IN, 3, 320], F32R)
            mb3 = consts.tile([C_IN, 3, C_OUT], F32)
            # seven PSUM banks as seven conv rotation slots
            yps = psyp.tile([C_OUT, 7, NT], F32)

            # PE warm-up: a burst of throwaway matmuls starting as soon
            # as the memset lands (~3us, before any DMA arrives) keeps
            # the tensor engine continuously busy so its p-state ramps
            # toward 2.4 GHz before the real conv stream begins.
            wsrc = consts.tile([2 * C_IN, NT], BF16)
            nc.gpsimd.memset(wsrc[:], 0.0)
            for _ in range(12):
                nc.tensor.matmul(yps[0:1, 0, 0:NT], wsrc[:, 0:1], wsrc[:],
                                 start=True, stop=True,
                                 skip_group_check=True)

            # software pipeline: stage(b) issues loads/copies, synth(b)
            # runs the small fp32r matmul chain, conv(b) the 16-tile
            # conv.  stage(b+2)'s loads are issued before conv(b) so its
            # DMAs sit ahead of conv(b)'s stores in the queues, but its
            # stats closures are dropped INTO conv(b)'s stream.
            stages = {}
            stages[0], _ = _emit_stage(nc, xp, small, scr, x_d, 0)
            nc.scalar.dma_start(m3[:], m3_d)
            nc.scalar.dma_start(mb3[:], mb3_d)
            tiles0, steps0 = _emit_synth_steps(nc, small, pss, m3, mb3,
                                               *stages[0])
            for s in steps0:           # sample 0: run synth immediately
                s()
            stages[1], _ = _emit_stage(nc, xp, small, scr, x_d, 1)
            synth = {0: tiles0}
            for b in range(BS):
                stats_cl = []
                if b + 2 < BS:
                    stages[b + 2], stats_cl = _emit_stage(
                        nc, xp, small, scr, x_d, b + 2, defer=True)
                if b + 1 < BS:
                    synth[b + 1], steps = _emit_synth_steps(
                        nc, small, pss, m3, mb3, *stages[b + 1])
                else:
                    steps = []
                # interleave: synth steps early (they gate the next
                # conv), stats sprinkled between them.
                closures = []
                si, wi = iter(steps), iter(stats_cl)
                order = "swswswswwwwwww"
                for ch in order:
                    c = next(si if ch == "s" else wi, None)
                    if c is not None:
                        closures.append(c)
                _emit_conv(nc, yp, yps, y_d, b, stages[b][0], *synth[b],
                           closures=closures)

    nc.compile()
    _CACHE["nc"] = nc
    return nc


def kernel(x, W_kernel, W_in, W_out, W_bias):
    x = np.asarray(x, dtype=np.float32)
    # one zero column each side: the device reads x[l-1], x[l], x[l+1]
    x = np.pad(x, [(0, 0), (0, 0), (1, 1)]).astype(ml_dtypes.bfloat16)
    m3, mb3 = _host_precompute(
        np.asarray(W_kernel, np.float32), np.asarray(W_in, np.float32),
        np.asarray(W_out, np.float32), np.asarray(W_bias, np.float32))

    nc = _build_module()
    in_maps = [
        {"x": x[c * BS:(c + 1) * BS], "m3": m3, "mb3": mb3}
        for c in range(N_CORES)
    ]
    res = run_bass_kernel_spmd(nc, in_maps, core_ids=list(range(N_CORES)))
    global LAST_RESULT
    LAST_RESULT = res
    y = np.concatenate([np.asarray(r["y"]) for r in res.results],
                       axis=0).astype(np.float32)
    return y


LAST_RESULT = None


# revision 21
# speedup vs baseline: 1.8918x; 1.8002x over previous
"""Trainium2 Bass kernel for nn_ConvPlus1d (dense_cnn).

Algorithm (mathematically identical to the reference, derived analytically):

  The reference synthesizes per-sample conv weights:
      kern[b]   = mean_L(depthwise_conv(x))        -> [B, C_IN, K]
      w_in[b]   = W_in @ kern[b]                   -> [B, C_IN, K]
      w_out[b]  = <W_out, kern[b]>                 -> [B, C_OUT]
      bias[b]   = <W_bias, kern[b]>                -> [B, C_OUT]
      weight[b, o, c, k] = w_in[b, c, k] * w_out[b, o]     (rank-1!)
      y[b] = conv1d(x[b], weight[b], pad=1) + bias[b]

  Exact simplifications:
  1) mean over L of a pad-1 depthwise conv only needs per-channel sums and
     the first/last elements:  sum_l xpad[c, l+t] = {S-E, S, S-F}[t]
     so kern / w_in / w_out / bias are LINEAR in (S, E, F), with
     coefficient matrices precomputed on the host from maker params.
  2) The per-sample conv weight is rank-1 across (o) x (c,k).

  Device program per sample (data-parallel over batch, 4 samples/core):
      x (bf16) lands in SBUF partitions 0-63; a shifted copy (one column
      left) is DMA'd into partitions 64-127.  The 3-tap conv then needs
      only TWO matmuls per 512-col tile: a 128-contract matmul computes
      taps 0+1 together (stationary [W0; W1]), a 64-contract matmul adds
      tap 2, k-outer over 4-tile groups so consecutive matmuls hit
      different PSUM banks.  Stats -> params synthesis runs in fp32r,
      interleaved into the previous sample's conv stream.  PSUM -> SBUF
      eviction adds the bias and narrows to bf16 (ACT only; DVE owns the
      stats reduces); the output is stored bf16 and widened to f32 on
      the host (~1e-3 extra relative error, half the store traffic);
      stores stream out in 2048-col chunks triggered from GpSimd.

Sharding: batch 32 -> 8 cores x 4 samples, maker params replicated.
"""

import sys

import numpy as np

sys.path.insert(0, "/opt/trn_rl_repo")

import concourse.bacc as bacc  # noqa: E402
import concourse.tile as tile  # noqa: E402
from concourse import mybir  # noqa: E402
from concourse.bass_utils import run_bass_kernel_spmd  # noqa: E402

import ml_dtypes  # noqa: E402

B, C_IN, C_OUT, K, L = 32, 64, 128, 3, 8192
N_CORES = 8
BS = B // N_CORES          # samples per core
NT = 512                   # matmul moving-dim tile (one PSUM bank of fp32)
NTILES = L // NT
NCH = 8                    # partial-reduce windows
CHW = (L + 2) // NCH       # 2048, last chunk takes the +2 remainder

F32 = mybir.dt.float32
F32R = mybir.dt.float32r
BF16 = mybir.dt.bfloat16


def _host_precompute(W_kernel, W_in, W_out, W_bias):
    """Fold the maker parameters into linear maps on the stats (S, E, F)."""
    Wk = W_kernel.reshape(C_IN, K, K).astype(np.float64)     # [c, j, t]
    P = (Wk[:, :, 0] + Wk[:, :, 1] + Wk[:, :, 2]) / L        # coeff on S
    Q = -Wk[:, :, 0] / L                                     # coeff on E
    R = -Wk[:, :, 2] / L                                     # coeff on F

    Win = W_in[:, :, 0].astype(np.float64)                   # [c, c']

    def m_in(Xc):   # -> [c', k*64+c]
        return np.einsum("cp,pk->pkc", Win, Xc).reshape(C_IN, K * C_IN)

    def m_out(Xc, W):  # -> [c', o]
        return np.einsum("ock,ck->co", W.astype(np.float64), Xc)

    def mm(Xc):
        return np.concatenate([m_in(Xc), m_out(Xc, W_out)], axis=1)  # [64,320]

    m3 = np.stack([mm(P), mm(Q), mm(R)], axis=1)             # [64, 3, 320]
    mb3 = np.stack(
        [m_out(P, W_bias), m_out(Q, W_bias), m_out(R, W_bias)], axis=1
    )                                                        # [64, 3, 128]
    return m3.astype(np.float32), mb3.astype(np.float32)


_CACHE = {}


def _emit_stage(nc, xp, small, x_d, b, trig=None):
    """Issue x load, shifted copy, and chunked stats for sample b.

    Loads and shifted copies use two big chunks (8KB per-partition rows:
    DMA queues are descriptor-rate limited, so fewer/fatter descriptors).
    Stats use four windows aligned to the load halves so each reduce
    depends on exactly one load chunk.
    """
    H = 4097                                 # load-chunk boundary
    xh = xp.tile([2 * C_IN, L + 2], BF16, tag="xh")
    Sp = small.tile([C_IN, NCH], F32, tag="Sp")
    trig = trig if trig is not None else nc.sync
    if b == 0:
        # sample 0 sits on the critical ramp: chunk the load/copy so the
        # stats reduces pipeline behind the DMA instead of after it
        lb = (0, 2048, H, 6144, L + 2)
        for c in range(4):
            trig.dma_start(xh[0:C_IN, lb[c]:lb[c + 1]],
                           x_d[b][:, lb[c]:lb[c + 1]])
        for c in range(4):
            d0, d1 = max(lb[c] - 1, 0), lb[c + 1] - 1
            trig.dma_start(xh[C_IN:, d0:d1], xh[0:C_IN, d0 + 1:d1 + 1])
        bnds = (0, 1024, 2048, 3072, H, 5120, 6144, 7169, L + 2)
    else:
        trig.dma_start(xh[0:C_IN, :], x_d[b])
        # dest col j <- src col j+1 (the shifted copy for taps 1/2)
        trig.dma_start(xh[C_IN:, 0:L + 1], xh[0:C_IN, 1:L + 2])
        bnds = (0, 1024, 2048, 3072, H, 5169, 6241, 7313, L + 2)
    # both zero-pad columns are included: they add nothing to S.
    for c in range(8):
        nc.vector.reduce_sum(out=Sp[:, c:c + 1],
                             in_=xh[0:C_IN, bnds[c]:bnds[c + 1]],
                             axis=mybir.AxisListType.X)
    return xh, Sp


def _emit_synth_steps(nc, small, pss, m3, mb3, xh, Sp):
    """Stats -> (w01, w2, biasv) for one sample, as four deferred steps.

    The steps are interleaved into the PREVIOUS sample's conv matmul
    stream so the PE <-> DVE ping-pong never drains the tensor engine
    (which would also drop its p-state).  Synth PSUM packs into two
    banks (disjoint address ranges, so interleaved accumulation groups
    are safe: skip_group_check).
    """
    stat = small.tile([C_IN, 3], F32R, tag="stat")
    syn_pb = pss.tile([C_OUT, 512], F32, tag="syn_pb")
    syn_w = pss.tile([2 * C_IN, 256], F32, tag="syn_w")
    psp, psb = syn_pb[0:1, 0:320], syn_pb[:, 320:321]
    ps01, ps2 = syn_w[:, 0:128], syn_w[C_IN:, 128:256]
    params = small.tile([1, 320], F32R, tag="params")
    biasv = small.tile([C_OUT, 1], F32, tag="biasv")
    w01 = small.tile([2 * C_IN, C_OUT], BF16, tag="w01")
    w2 = small.tile([2 * C_IN, C_OUT], BF16, tag="w2")

    def step0():   # stats gather (DVE) + stat matmuls (PE)
        # fp32r is 32-bit in SBUF: the low-precision guard is a false alarm
        with nc.allow_low_precision(reason="fp32r out is fp32 bits"):
            nc.vector.reduce_sum(out=stat[:, 0:1], in_=Sp[:],
                                 axis=mybir.AxisListType.X)
        nc.vector.tensor_copy(stat[:, 1:2], xh[0:C_IN, L:L + 1])   # E
        nc.vector.tensor_copy(stat[:, 2:3], xh[0:C_IN, 1:2])       # F
        for j in range(3):
            sj = stat[:, j:j + 1]
            nc.tensor.matmul(psp, sj, m3[:, j, :], start=(j == 0),
                             stop=(j == 2), skip_group_check=True)
            # 1 moving column: fp32 4-pass costs nothing, and fp32r
            # moving free-size 1 fails the ISA check
            nc.tensor.matmul(psb, mb3[:, j, :], sj.bitcast(F32),
                             start=(j == 0), stop=(j == 2),
                             skip_group_check=True)

    def step1():
        nc.vector.tensor_copy(params[:], psp)
        nc.vector.tensor_copy(biasv[:], psb)

    def step2():
        # rank-1 stationaries: [W0; W1] on partitions 0-127, W2 on
        # 64-127.  contract-1 outers: fp32r fails the ISA check, fp32
        # 4-pass on 128 moving cols is well under a microsecond.
        pr = params[0:1].bitcast(F32)
        w_out_row = pr[:, 192:320]
        nc.tensor.matmul(ps01, pr[:, 0:128], w_out_row, start=True,
                         stop=True, skip_group_check=True)
        nc.tensor.matmul(ps2, pr[:, 128:192], w_out_row, start=True,
                         stop=True, skip_group_check=True)

    def step3():
        nc.vector.tensor_copy(w01[:], ps01)
        nc.vector.tensor_copy(w2[C_IN:, :], ps2)

    return (w01, w2, biasv), [step0, step1, step2, step3]


def _emit_conv(nc, yp, psy, y_d, b, xh, w01, w2, biasv, steps=()):
    """Main conv for one sample: 16 tiles x (2 matmuls, evict); 2048-col
    store chunks.  `steps` are the next sample's synth stages, dropped
    into the instruction stream mid-conv."""
    SCW = 4 * NT                             # store-chunk columns
    for g in range(NTILES // 4):
        ysb = yp.tile([C_OUT, SCW], BF16, tag="ysb")
        yo = 0
        pys = []
        # k-outer within the group: consecutive matmuls hit DIFFERENT
        # PSUM banks, so the mm2-accumulates-onto-mm1 dependency sits 4
        # matmuls back and the PE pipeline never drains (same-bank
        # back-to-back accumulation costs ~100 extra cycles per matmul).
        for i in range(4):
            py = psy.tile([C_OUT, NT], F32, tag="py")
            pys.append(py)
            m = NT * (4 * g + i)
            nc.tensor.matmul(py[:], w01[:], xh[:, m:m + NT],
                             start=True, stop=False)
        for i in range(4):
            m = NT * (4 * g + i)
            nc.tensor.matmul(pys[i][:], w2[C_IN:, :],
                             xh[C_IN:, m + 1:m + NT + 1],
                             start=False, stop=True)
        # evictions live on ACT alone: DVE holds the 8.8us/sample stats
        # reduces, and any eviction queued behind them stalls the PE on
        # PSUM banks.
        for i in range(4):
            nc.scalar.activation(ysb[:, yo + i * NT:yo + (i + 1) * NT],
                                 pys[i][:],
                                 mybir.ActivationFunctionType.Identity,
                                 bias=biasv[:], scale=1.0)
            # the last sample's stores are the kernel tail: drain each
            # half-chunk as soon as its evictions land
            if b == BS - 1 and i % 2 == 1:
                h0 = 4 * g * NT + (i - 1) * NT
                nc.gpsimd.dma_start(y_d[b][:, h0:h0 + 2 * NT],
                                    ysb[:, (i - 1) * NT:(i + 1) * NT])
        if b != BS - 1:
            nc.gpsimd.dma_start(y_d[b][:, 4 * g * NT:4 * g * NT + SCW],
                                ysb[:])
        if g < len(steps):
            steps[g]()


def _build_module():
    if "nc" in _CACHE:
        return _CACHE["nc"]
    nc = bacc.Bacc("TRN2", target_bir_lowering=False, debug=False)

    # host supplies x pre-padded with one zero column on each side, bf16
    x_d = nc.dram_tensor("x", [BS, C_IN, L + 2], BF16,
                         kind="ExternalInput").ap()
    m3_d = nc.dram_tensor("m3", [C_IN, 3, 320], F32R,
                          kind="ExternalInput").ap()
    mb3_d = nc.dram_tensor("mb3", [C_IN, 3, C_OUT], F32,
                           kind="ExternalInput").ap()
    y_d = nc.dram_tensor("y", [BS, C_OUT, L], BF16,
                         kind="ExternalOutput").ap()

    with tile.TileContext(nc) as tc:
        with (
            tc.tile_pool(name="consts", bufs=1) as consts,
            tc.tile_pool(name="xp", bufs=4) as xp,
            tc.tile_pool(name="yp", bufs=6) as yp,
            tc.tile_pool(name="small", bufs=2) as small,
            tc.tile_pool(name="ps_y", bufs=6, space="PSUM") as psy,
            tc.tile_pool(name="ps_s", bufs=1, space="PSUM") as pss,
        ):
            m3 = consts.tile([C_IN, 3, 320], F32R)
            mb3 = consts.tile([C_IN, 3, C_OUT], F32)

            # software pipeline: stage(b) issues loads/copies/stats, synth(b)
            # runs the small fp32r matmul chain, conv(b) the 16-tile conv.
            # stage(b+2) is issued before conv(b) so its DMAs sit ahead of
            # conv(b)'s stores in the queues; stats(b+1)/(b+2) sit ahead of
            # conv(b)'s DVE evictions.  Sample 0's x load is issued before
            # the consts so it is the first transfer in the queues; consts
            # trigger from the Scalar engine to spread descriptor-gen.
            stages = {}
            stages[0] = _emit_stage(nc, xp, small, x_d, 0)
            nc.scalar.dma_start(m3[:], m3_d)
            nc.scalar.dma_start(mb3[:], mb3_d)
            tiles0, steps0 = _emit_synth_steps(nc, small, pss, m3, mb3,
                                               *stages[0])
            for s in steps0:           # sample 0: run synth immediately
                s()
            stages[1] = _emit_stage(nc, xp, small, x_d, 1)
            synth = {0: tiles0}
            for b in range(BS):
                if b + 2 < BS:
                    stages[b + 2] = _emit_stage(nc, xp, small, x_d, b + 2)
                if b + 1 < BS:
                    synth[b + 1], nxt_steps = _emit_synth_steps(
                        nc, small, pss, m3, mb3, *stages[b + 1])
                else:
                    nxt_steps = ()
                _emit_conv(nc, yp, psy, y_d, b, stages[b][0], *synth[b],
                           steps=nxt_steps)

    nc.compile()
    _CACHE["nc"] = nc
    return nc


def kernel(x, W_kernel, W_in, W_out, W_bias):
    x = np.asarray(x, dtype=np.float32)
    # one zero column each side: the device reads x[l-1], x[l], x[l+1]
    x = np.pad(x, [(0, 0), (0, 0), (1, 1)]).astype(ml_dtypes.bfloat16)
    m3, mb3 = _host_precompute(
        np.asarray(W_kernel, np.float32), np.asarray(W_in, np.float32),
        np.asarray(W_out, np.float32), np.asarray(W_bias, np.float32))

    nc = _build_module()
    in_maps = [
        {"x": x[c * BS:(c + 1) * BS], "m3": m3, "mb3": mb3}
        for c in range(N_CORES)
    ]
    res = run_bass_kernel_spmd(nc, in_maps, core_ids=list(range(N_CORES)))
    global LAST_RESULT
    LAST_RESULT = res
    y = np.concatenate([np.asarray(r["y"]) for r in res.results],
                       axis=0).astype(np.float32)
    return y


LAST_RESULT = None

